# revision 18
# baseline (speedup 1.0000x reference)
"""MIHash loss kernel for Trainium2 (8 NeuronCores, SPMD) — v3.

Math: loss = sum_i ent(pD_i) - prCp_i*ent(pDCp_i) - prCn_i*ent(pDCn_i),
histograms from triangular pulses of w = dist/4 = 8 - phi_i.phi_j/8,
hat(w-b) for b = 0..15.  With off-diagonal w in (6,10) (validated), the
16 bins derive from R(c) = sum_j relu(w_ij - c) at c = 7, 8, 9 plus the
exact row sum T (host, fp64) via H[b] = R(b-1) - 2R(b) + R(b+1) with
linear fills (R(c) = T - cN for c <= 6) and zeros (c >= 10).

Device (per core: 1024 rows of the label-sorted problem, 8 blocks of
128): the 8192 columns are processed in 4 supergroups of 2048, each
split into two 1024-col PSUM tiles — one consumed by ACT, one by DVE.
Cross-engine reads of one PSUM tile serialize on TRN2, so each engine
owns private tiles and the two reduction pipelines run independently:
  ACT (left half):  relu((8-c) - p/8) + accum            -> R-part
  DVE (right half): min(p, 8(8-c)) + accum, /8 on host   -> S-part
The same-class (xp) side uses a [128, win] band around the diagonal of
the sorted order (window from a per-core bandT input): 3 DVE STT taps
sum M * min(p, 8(8-c)) where M is a 0/1 segment mask (diag excluded);
R_p(c) = (8-c)*Sp - S/8.

Host (fp64): sort by label, T / Tp / diagonal handled exactly (no range
assumption on the diagonal), validation of the off-diagonal range from
R(7), R(9) against host expectations (fallback to numpy on violation),
second differences, entropies.
"""

import numpy as np

import concourse.bass as bass
import concourse.mybir as mybir
import concourse.tile as tile
from concourse import bacc
from concourse.bass_utils import run_bass_kernel_spmd

N = 8192
NBIT = 64
NCORES = 8
ROWS_PER_CORE = N // NCORES          # 1024
BLOCKS = ROWS_PER_CORE // 128        # 8
NBINS = 16
EPS = 1e-7

SG = 2048                            # supergroup width
NSG = N // SG                        # 4 per block
HW_ = SG // 2                        # 1024 cols per engine per supergroup
TAPS = (7, 8, 9)

F32 = mybir.dt.float32
F16 = mybir.dt.float16

_PROGRAM_CACHE = {}

# racc column layout per block (32 cols):
#   ti*8 + sg*2 + 0 : ACT relu-sum, cols [sg*2048, +1024)
#   ti*8 + sg*2 + 1 : DVE min-sum*8, cols [sg*2048+1024, +1024)
#   24+ti           : band min-sum over q
NCOL = 32


def _build_program(pad: int):
    win = 128 + 2 * pad
    bw = ROWS_PER_CORE + 2 * pad

    nc = bacc.Bacc(
        "TRN2", target_bir_lowering=False, debug=False, num_devices=NCORES
    )
    phiT_d = nc.dram_tensor("phiT", [NBIT, N], F16, kind="ExternalInput")
    bandT_d = nc.dram_tensor("bandT", [NBIT, bw], F16, kind="ExternalInput")
    amask_d = nc.dram_tensor("amask", [BLOCKS, 128, win], F16, kind="ExternalInput")
    racc_d = nc.dram_tensor("racc", [BLOCKS, 128, NCOL], F32, kind="ExternalOutput")

    mn = mybir.AluOpType.min
    add = mybir.AluOpType.add
    mult = mybir.AluOpType.mult
    relu = mybir.ActivationFunctionType.Relu

    with tile.TileContext(nc) as tc:
        with (
            tc.tile_pool(name="const", bufs=1) as constp,
            tc.tile_pool(name="ascr", bufs=2) as ascrp,
            tc.tile_pool(name="dscr", bufs=2) as dscrp,
            tc.tile_pool(name="bq", bufs=2) as bqp,
            tc.tile_pool(name="acc", bufs=1) as accp,
            tc.tile_pool(name="psA", bufs=2, space=bass.MemorySpace.PSUM) as psA,
            tc.tile_pool(name="psD", bufs=2, space=bass.MemorySpace.PSUM) as psD,
        ):
            phiT = constp.tile([NBIT, N], F16)
            bandT = constp.tile([NBIT, bw], F16)
            amask = constp.tile([128, BLOCKS * win], F16)
            nc.sync.dma_start(bandT[:], bandT_d[:])
            for g in range(NSG):
                nc.sync.dma_start(
                    phiT[:, SG * g : SG * (g + 1)], phiT_d[:, SG * g : SG * (g + 1)]
                )
            # one strided DMA for all block masks: dram [b][p][x] -> sbuf [p][b*win+x]
            nc.sync.dma_start(
                amask[:].rearrange("p (b x) -> p b x", b=BLOCKS),
                amask_d[:].transpose([1, 0, 2]),
            )

            racc_s = accp.tile([128, BLOCKS * NCOL], F32)
            nc.vector.memset(racc_s[:], 0.0)

            biases = constp.tile([128, len(TAPS)], F32)
            bias_col = {}
            for ti, c in enumerate(TAPS):
                nc.vector.memset(biases[:, ti : ti + 1], float(8 - c))
                bias_col[c] = biases[:, ti : ti + 1]

            # warm the ACT function table early (overlaps input DMA)
            warm = constp.tile([128, 1], F32)
            nc.vector.memset(warm[:], 0.0)
            wsc = constp.tile([128, 1], F32)
            nc.scalar.activation(wsc[:], warm[:], relu, bias=bias_col[8], scale=1.0)

            for blk in range(BLOCKS):
                own = bandT[:, pad + 128 * blk : pad + 128 * (blk + 1)]
                r0 = blk * NCOL

                def supergroup(sg, r0=r0, own=own, blk=blk):
                    base = SG * sg
                    ppa = psA.tile([128, HW_], F32, tag="ppA")
                    ppd = psD.tile([128, HW_], F32, tag="ppD")
                    for s in range(HW_ // 512):
                        nc.tensor.matmul(
                            ppa[:, 512 * s : 512 * (s + 1)],
                            own,
                            phiT[:, base + 512 * s : base + 512 * (s + 1)],
                            start=True,
                            stop=True,
                        ).annotate(f"mmA_b{blk}s{sg}")
                    for s in range(HW_ // 512):
                        nc.tensor.matmul(
                            ppd[:, 512 * s : 512 * (s + 1)],
                            own,
                            phiT[:, base + HW_ + 512 * s : base + HW_ + 512 * (s + 1)],
                            start=True,
                            stop=True,
                        ).annotate(f"mmD_b{blk}s{sg}")
                    for ti, c in enumerate(TAPS):
                        a = float(8 - c)
                        scr = ascrp.tile([128, HW_], F32, tag="as")
                        nc.scalar.activation(
                            scr[:], ppa[:], relu, bias=bias_col[c], scale=-0.125,
                            accum_out=racc_s[:, r0 + 8 * ti + 2 * sg : r0 + 8 * ti + 2 * sg + 1],
                        ).annotate(f"tapA_b{blk}s{sg}c{c}")
                        scr2 = dscrp.tile([128, HW_], F32, tag="ds")
                        nc.vector.tensor_scalar(
                            scr2[:], ppd[:], 8.0 * a, None, mn, add,
                            accum_out=racc_s[:, r0 + 8 * ti + 2 * sg + 1 : r0 + 8 * ti + 2 * sg + 2],
                        ).annotate(f"tapD_b{blk}s{sg}c{c}")

                def band_work(blk=blk, r0=r0, own=own):
                    ppb = psD.tile([128, HW_], F32, tag="ppD")
                    off = 0
                    while off < win:
                        cw = min(512, win - off)
                        nc.tensor.matmul(
                            ppb[:, off : off + cw],
                            own,
                            bandT[:, 128 * blk + off : 128 * blk + off + cw],
                            start=True,
                            stop=True,
                        ).annotate(f"mmband_b{blk}")
                        off += cw
                    # tap = sum_j M_j * min(p_j, 8a) = 8 * sum_seg min(t, a)
                    for ti, c in enumerate(TAPS):
                        a = float(8 - c)
                        scr = bqp.tile([128, win], F32, tag="bs")
                        nc.vector.scalar_tensor_tensor(
                            scr[:], ppb[:, 0:win], 8.0 * a,
                            amask[:, blk * win : (blk + 1) * win], mn, mult,
                            accum_out=racc_s[:, r0 + 24 + ti : r0 + 25 + ti],
                        ).annotate(f"tapB_b{blk}c{c}")

                supergroup(0)
                supergroup(1)
                band_work()
                supergroup(2)
                supergroup(3)

            for blk in range(BLOCKS):
                nc.sync.dma_start(
                    racc_d[blk], racc_s[:, blk * NCOL : (blk + 1) * NCOL]
                )

    nc.compile()
    return nc, win, bw


class _RangeViolation(Exception):
    pass


def _numpy_reference(u, y):
    """Exact fp64 fallback (non-one-hot y or off-diagonal range violation)."""
    u = u.astype(np.float64)
    y = y.astype(np.float64)
    n, nbits = u.shape
    aff = ((y @ y.T) > 0).astype(np.float64)
    np.fill_diagonal(aff, 0.0)
    xp = aff
    xn = 1.0 - aff
    phi = 2.0 / (1.0 + np.exp(-u)) - 1.0
    dist = (nbits - phi @ phi.T) * 0.5
    prCp = xp.sum(1) / (n - 1)
    prCn = 1.0 - prCp
    delta = nbits // NBINS
    pDCp = np.zeros((n, NBINS))
    pDCn = np.zeros((n, NBINS))
    for b in range(NBINS):
        mid = b * delta
        ind = (dist > mid - delta) & (dist <= mid + delta)
        pulse = np.where(ind, 1.0 - np.abs(dist - mid) / delta, 0.0)
        pDCp[:, b] = (pulse * xp).sum(1)
        pDCn[:, b] = (pulse * xn).sum(1)
    return _finish_loss(pDCp, pDCn, prCp, prCn, n)


def _finish_loss(pDCp, pDCn, prCp, prCn, n):
    pD = (pDCp + pDCn) / (n - 1)
    sum_p = pDCp.sum(1)
    sum_n = pDCn.sum(1)
    safe_p = np.where(sum_p > 0, sum_p, 1.0)
    safe_n = np.where(sum_n > 0, sum_n, 1.0)
    pDCp = np.where((sum_p > 0)[:, None], pDCp / safe_p[:, None], pDCp)
    pDCn = np.where((sum_n > 0)[:, None], pDCn / safe_n[:, None], pDCn)

    def ent(p):
        return -(p * np.log(p + EPS)).sum(1)

    loss = (ent(pD) - (prCp * ent(pDCp) + prCn * ent(pDCn))).sum()
    return np.array(loss, dtype=np.float32)


def _hat(x):
    return np.maximum(0.0, 1.0 - np.abs(x))


_LAST_RESULTS = None


def kernel(u, y):
    u = np.ascontiguousarray(np.asarray(u), dtype=np.float32)
    y = np.asarray(y)
    assert u.shape == (N, NBIT)

    pos = y > 0
    if not (pos.sum(axis=1) == 1).all() or (y < 0).any():
        return _numpy_reference(u, np.asarray(y, np.float32))
    labels = pos.argmax(axis=1)

    perm = np.argsort(labels, kind="stable")
    labels_s = labels[perm]
    counts = np.bincount(labels_s, minlength=labels_s.max() + 1)
    starts = np.concatenate([[0], np.cumsum(counts)])
    seg_s = starts[labels_s]
    seg_e = starts[labels_s + 1]
    maxn = int(counts.max())

    pad = 128
    while maxn - 1 > pad:
        pad += 128
    win = 128 + 2 * pad
    bw = ROWS_PER_CORE + 2 * pad

    if pad not in _PROGRAM_CACHE:
        _PROGRAM_CACHE[pad] = _build_program(pad)
    nc, win_, bw_ = _PROGRAM_CACHE[pad]
    assert (win_, bw_) == (win, bw)

    phi16 = np.tanh(u / 2.0).astype(np.float16)
    phiT = np.ascontiguousarray(phi16[perm].T)           # [64, N] f16, sorted
    phi64 = phiT.T.astype(np.float64)

    s_all = phi64.sum(axis=0)
    t_row = (phi64 @ s_all) / 8.0                        # sum_j t_ij incl diag
    t_diag = (phi64 * phi64).sum(axis=1) / 8.0
    w_diag = 8.0 - t_diag
    T_all = 8.0 * N - t_row

    ncls = len(counts)
    cls_sums = np.zeros((ncls, NBIT))
    np.add.at(cls_sums, labels_s, phi64)
    nseg = (seg_e - seg_s).astype(np.float64)
    Sp = nseg - 1.0
    Tp = 8.0 * Sp - ((phi64 * (cls_sums[labels_s] - phi64)).sum(axis=1)) / 8.0

    in_maps = []
    for core in range(NCORES):
        off = core * ROWS_PER_CORE
        lo = off - pad
        band = np.zeros((NBIT, bw), dtype=np.float16)
        c0 = max(0, lo)
        c1 = min(N, off + ROWS_PER_CORE + pad)
        band[:, c0 - lo : c1 - lo] = phiT[:, c0:c1]

        am = np.zeros((BLOCKS, 128, win), dtype=np.float16)
        idx = np.arange(win)[None, :]
        for blk in range(BLOCKS):
            w0 = off + 128 * blk - pad
            rows = np.arange(off + 128 * blk, off + 128 * (blk + 1))
            xs = seg_s[rows] - w0
            xe = seg_e[rows] - w0
            assert (xs >= 0).all() and (xe <= win).all(), "segment outside window"
            inside = (idx >= xs[:, None]) & (idx < xe[:, None])
            am[blk][inside] = 1.0
            am[blk, np.arange(128), rows - w0] = 0.0     # exclude diagonal
        in_maps.append({"phiT": phiT, "bandT": band, "amask": am})

    try:
        return _postprocess_and_loss(
            nc, in_maps, seg_s, seg_e, pad, T_all, Tp, Sp, w_diag
        )
    except _RangeViolation:
        return _numpy_reference(u, np.asarray(y, np.float32))


def _postprocess_and_loss(nc, in_maps, seg_s, seg_e, pad, T_all, Tp, Sp, w_diag):
    global _LAST_RESULTS
    res = run_bass_kernel_spmd(nc, in_maps, list(range(NCORES)))
    _LAST_RESULTS = res

    win = 128 + 2 * pad
    pDCp = np.zeros((N, NBINS))
    pDCn = np.zeros((N, NBINS))
    for core in range(NCORES):
        out = res.results[core]
        racc = out["racc"].astype(np.float64)            # [8, 128, 32]
        off = core * ROWS_PER_CORE
        rows = np.arange(off, off + ROWS_PER_CORE)

        R_all = np.zeros((ROWS_PER_CORE, 3))
        for ti, c in enumerate(TAPS):
            a = float(8 - c)
            acc = np.zeros(ROWS_PER_CORE)
            for sg in range(NSG):
                acc += racc[:, :, 8 * ti + 2 * sg].reshape(-1)          # ACT
                acc += a * HW_ - racc[:, :, 8 * ti + 2 * sg + 1].reshape(-1) / 8.0
            R_all[:, ti] = acc

        wd = w_diag[rows]
        Td = T_all[rows]
        L7 = R_all[:, 0] + 7.0 * N - Td                  # sum relu(7-w) incl diag
        exc7 = L7 - np.maximum(7.0 - wd, 0.0)
        R9p = R_all[:, 2] - np.maximum(wd - 9.0, 0.0)
        if (exc7 > 0.5).any() or (R9p > 0.5).any() or (exc7 < -0.5).any():
            raise _RangeViolation()

        Rt = np.zeros((ROWS_PER_CORE, 18))               # c = -1 .. 16
        Tdp = Td - wd
        for c in range(-1, 7):
            Rt[:, c + 1] = Tdp - float(c) * (N - 1)
        for ti, c in enumerate(TAPS):
            Rt[:, c + 1] = R_all[:, ti] - np.maximum(wd - c, 0.0)
        H_all = Rt[:, 0:16] - 2.0 * Rt[:, 1:17] + Rt[:, 2:18]
        H_all[:, :6] = 0.0
        H_all[:, 11:] = 0.0
        H_all = np.maximum(H_all, 0.0)

        Rb = np.zeros((ROWS_PER_CORE, 18))
        Spr = Sp[rows]
        Tpr = Tp[rows]
        for c in range(-1, 7):
            Rb[:, c + 1] = Tpr - float(c) * Spr
        for ti, c in enumerate(TAPS):
            a = float(8 - c)
            Rb[:, c + 1] = a * Spr - racc[:, :, 24 + ti].reshape(-1) / 8.0
        H_p = Rb[:, 0:16] - 2.0 * Rb[:, 1:17] + Rb[:, 2:18]
        H_p[:, :6] = 0.0
        H_p[:, 11:] = 0.0
        H_p = np.maximum(H_p, 0.0)

        H_n = np.maximum(H_all - H_p, 0.0)
        bins = np.arange(NBINS)[None, :]
        H_n += _hat(wd[:, None] - bins)                  # diagonal (xn_ii = 1)
        pDCp[rows] = H_p
        pDCn[rows] = H_n

    prCp = Sp / (N - 1)
    prCn = 1.0 - prCp
    return _finish_loss(pDCp, pDCn, prCp, prCn, N)


# revision 20
# speedup vs baseline: 1.0351x; 1.0351x over previous
"""MIHash loss kernel for Trainium2 (8 NeuronCores, SPMD) — v3.

Math: loss = sum_i ent(pD_i) - prCp_i*ent(pDCp_i) - prCn_i*ent(pDCn_i),
histograms from triangular pulses of w = dist/4 = 8 - phi_i.phi_j/8,
hat(w-b) for b = 0..15.  With off-diagonal w in (6,10) (validated), the
16 bins derive from R(c) = sum_j relu(w_ij - c) at c = 7, 8, 9 plus the
exact row sum T (host, fp64) via H[b] = R(b-1) - 2R(b) + R(b+1) with
linear fills (R(c) = T - cN for c <= 6) and zeros (c >= 10).

Device (per core: 1024 rows of the label-sorted problem, 8 blocks of
128): the 8192 columns are processed in 4 supergroups of 2048, each
split into two 1024-col PSUM tiles — one consumed by ACT, one by DVE.
Cross-engine reads of one PSUM tile serialize on TRN2, so each engine
owns private tiles and the two reduction pipelines run independently:
  ACT (left half):  relu((8-c) - p/8) + accum            -> R-part
  DVE (right half): min(p, 8(8-c)) + accum, /8 on host   -> S-part
The same-class (xp) side uses a [128, win] band around the diagonal of
the sorted order (window from a per-core bandT input): 3 DVE STT taps
sum M * min(p, 8(8-c)) where M is a 0/1 segment mask (diag excluded);
R_p(c) = (8-c)*Sp - S/8.

Host (fp64): sort by label, T / Tp / diagonal handled exactly (no range
assumption on the diagonal), validation of the off-diagonal range from
R(7), R(9) against host expectations (fallback to numpy on violation),
second differences, entropies.
"""

import numpy as np

import concourse.bass as bass
import concourse.mybir as mybir
import concourse.tile as tile
from concourse import bacc
from concourse.bass_utils import run_bass_kernel_spmd

N = 8192
NBIT = 64
NCORES = 8
ROWS_PER_CORE = N // NCORES          # 1024
BLOCKS = ROWS_PER_CORE // 128        # 8
NBINS = 16
EPS = 1e-7

SG = 2048                            # supergroup width
NSG = N // SG                        # 4 per block
HW_ = SG // 2                        # 1024 cols per engine per supergroup
TAPS = (7, 8, 9)

F32 = mybir.dt.float32
F16 = mybir.dt.float16

_PROGRAM_CACHE = {}

# racc column layout per block (32 cols):
#   ti*8 + sg*2 + 0 : ACT relu-sum, cols [sg*2048, +1024)
#   ti*8 + sg*2 + 1 : DVE min-sum*8, cols [sg*2048+1024, +1024)
#   24+ti           : band min-sum over q
NCOL = 32


def _build_program(pad: int):
    win = 128 + 2 * pad
    bw = ROWS_PER_CORE + 2 * pad

    nc = bacc.Bacc(
        "TRN2", target_bir_lowering=False, debug=False, num_devices=NCORES
    )
    phiT_d = nc.dram_tensor("phiT", [NBIT, N], F16, kind="ExternalInput")
    bandT_d = nc.dram_tensor("bandT", [NBIT, bw], F16, kind="ExternalInput")
    amask_d = nc.dram_tensor("amask", [BLOCKS, 128, win], F16, kind="ExternalInput")
    racc_d = nc.dram_tensor("racc", [BLOCKS, 128, NCOL], F32, kind="ExternalOutput")

    mn = mybir.AluOpType.min
    add = mybir.AluOpType.add
    mult = mybir.AluOpType.mult
    relu = mybir.ActivationFunctionType.Relu

    with tile.TileContext(nc) as tc:
        with (
            tc.tile_pool(name="const", bufs=1) as constp,
            tc.tile_pool(name="ascr", bufs=2) as ascrp,
            tc.tile_pool(name="dscr", bufs=2) as dscrp,
            tc.tile_pool(name="bq", bufs=2) as bqp,
            tc.tile_pool(name="acc", bufs=1) as accp,
            tc.tile_pool(name="psA", bufs=2, space=bass.MemorySpace.PSUM) as psA,
            tc.tile_pool(name="psD", bufs=2, space=bass.MemorySpace.PSUM) as psD,
        ):
            phiT0 = constp.tile([NBIT, SG], F16)
            phiTr = constp.tile([NBIT, N - SG], F16)
            bandT = constp.tile([NBIT, bw], F16)
            amask = constp.tile([128, BLOCKS * win], F16)
            # parallel DMA queues so the first matmul starts early: bandT on
            # SP, phiT (first supergroup as its own tile) on ACT, masks on DVE
            nc.sync.dma_start(bandT[:], bandT_d[:])
            nc.scalar.dma_start(phiT0[:], phiT_d[:, 0:SG])
            nc.scalar.dma_start(phiTr[:], phiT_d[:, SG:N])
            # one strided DMA for all block masks: dram [b][p][x] -> sbuf [p][b*win+x]
            nc.gpsimd.dma_start(
                amask[:].rearrange("p (b x) -> p b x", b=BLOCKS),
                amask_d[:].transpose([1, 0, 2]),
            )

            def phi_cols(c0, c1):
                if c1 <= SG:
                    return phiT0[:, c0:c1]
                assert c0 >= SG
                return phiTr[:, c0 - SG : c1 - SG]

            racc_s = accp.tile([128, BLOCKS * NCOL], F32)
            nc.vector.memset(racc_s[:], 0.0)

            biases = constp.tile([128, len(TAPS)], F32)
            bias_col = {}
            for ti, c in enumerate(TAPS):
                nc.vector.memset(biases[:, ti : ti + 1], float(8 - c))
                bias_col[c] = biases[:, ti : ti + 1]

            # warm the ACT function table early (overlaps input DMA)
            warm = constp.tile([128, 1], F32)
            nc.vector.memset(warm[:], 0.0)
            wsc = constp.tile([128, 1], F32)
            nc.scalar.activation(wsc[:], warm[:], relu, bias=bias_col[8], scale=1.0)

            for blk in range(BLOCKS):
                own = bandT[:, pad + 128 * blk : pad + 128 * (blk + 1)]
                r0 = blk * NCOL

                def supergroup(sg, r0=r0, own=own, blk=blk):
                    base = SG * sg
                    ppa = psA.tile([128, HW_], F32, tag="ppA")
                    ppd = psD.tile([128, HW_], F32, tag="ppD")
                    for s in range(HW_ // 512):
                        nc.tensor.matmul(
                            ppa[:, 512 * s : 512 * (s + 1)],
                            own,
                            phi_cols(base + 512 * s, base + 512 * (s + 1)),
                            start=True,
                            stop=True,
                        ).annotate(f"mmA_b{blk}s{sg}")
                    for s in range(HW_ // 512):
                        nc.tensor.matmul(
                            ppd[:, 512 * s : 512 * (s + 1)],
                            own,
                            phi_cols(base + HW_ + 512 * s, base + HW_ + 512 * (s + 1)),
                            start=True,
                            stop=True,
                        ).annotate(f"mmD_b{blk}s{sg}")
                    for ti, c in enumerate(TAPS):
                        a = float(8 - c)
                        scr = ascrp.tile([128, HW_], F32, tag="as")
                        nc.scalar.activation(
                            scr[:], ppa[:], relu, bias=bias_col[c], scale=-0.125,
                            accum_out=racc_s[:, r0 + 8 * ti + 2 * sg : r0 + 8 * ti + 2 * sg + 1],
                        ).annotate(f"tapA_b{blk}s{sg}c{c}")
                        scr2 = dscrp.tile([128, HW_], F32, tag="ds")
                        nc.vector.tensor_scalar(
                            scr2[:], ppd[:], 8.0 * a, None, mn, add,
                            accum_out=racc_s[:, r0 + 8 * ti + 2 * sg + 1 : r0 + 8 * ti + 2 * sg + 2],
                        ).annotate(f"tapD_b{blk}s{sg}c{c}")

                def band_work(blk=blk, r0=r0, own=own):
                    ppb = psD.tile([128, HW_], F32, tag="ppD")
                    off = 0
                    while off < win:
                        cw = min(512, win - off)
                        nc.tensor.matmul(
                            ppb[:, off : off + cw],
                            own,
                            bandT[:, 128 * blk + off : 128 * blk + off + cw],
                            start=True,
                            stop=True,
                        ).annotate(f"mmband_b{blk}")
                        off += cw
                    # tap = sum_j M_j * min(p_j, 8a) = 8 * sum_seg min(t, a)
                    for ti, c in enumerate(TAPS):
                        a = float(8 - c)
                        scr = bqp.tile([128, win], F32, tag="bs")
                        nc.vector.scalar_tensor_tensor(
                            scr[:], ppb[:, 0:win], 8.0 * a,
                            amask[:, blk * win : (blk + 1) * win], mn, mult,
                            accum_out=racc_s[:, r0 + 24 + ti : r0 + 25 + ti],
                        ).annotate(f"tapB_b{blk}c{c}")

                supergroup(0)
                supergroup(1)
                band_work()
                supergroup(2)
                supergroup(3)

            for blk in range(BLOCKS):
                nc.sync.dma_start(
                    racc_d[blk], racc_s[:, blk * NCOL : (blk + 1) * NCOL]
                )

    nc.compile()
    return nc, win, bw


class _RangeViolation(Exception):
    pass


def _numpy_reference(u, y):
    """Exact fp64 fallback (non-one-hot y or off-diagonal range violation)."""
    u = u.astype(np.float64)
    y = y.astype(np.float64)
    n, nbits = u.shape
    aff = ((y @ y.T) > 0).astype(np.float64)
    np.fill_diagonal(aff, 0.0)
    xp = aff
    xn = 1.0 - aff
    phi = 2.0 / (1.0 + np.exp(-u)) - 1.0
    dist = (nbits - phi @ phi.T) * 0.5
    prCp = xp.sum(1) / (n - 1)
    prCn = 1.0 - prCp
    delta = nbits // NBINS
    pDCp = np.zeros((n, NBINS))
    pDCn = np.zeros((n, NBINS))
    for b in range(NBINS):
        mid = b * delta
        ind = (dist > mid - delta) & (dist <= mid + delta)
        pulse = np.where(ind, 1.0 - np.abs(dist - mid) / delta, 0.0)
        pDCp[:, b] = (pulse * xp).sum(1)
        pDCn[:, b] = (pulse * xn).sum(1)
    return _finish_loss(pDCp, pDCn, prCp, prCn, n)


def _finish_loss(pDCp, pDCn, prCp, prCn, n):
    pD = (pDCp + pDCn) / (n - 1)
    sum_p = pDCp.sum(1)
    sum_n = pDCn.sum(1)
    safe_p = np.where(sum_p > 0, sum_p, 1.0)
    safe_n = np.where(sum_n > 0, sum_n, 1.0)
    pDCp = np.where((sum_p > 0)[:, None], pDCp / safe_p[:, None], pDCp)
    pDCn = np.where((sum_n > 0)[:, None], pDCn / safe_n[:, None], pDCn)

    def ent(p):
        return -(p * np.log(p + EPS)).sum(1)

    loss = (ent(pD) - (prCp * ent(pDCp) + prCn * ent(pDCn))).sum()
    return np.array(loss, dtype=np.float32)


def _hat(x):
    return np.maximum(0.0, 1.0 - np.abs(x))


_LAST_RESULTS = None


def kernel(u, y):
    u = np.ascontiguousarray(np.asarray(u), dtype=np.float32)
    y = np.asarray(y)
    assert u.shape == (N, NBIT)

    pos = y > 0
    if not (pos.sum(axis=1) == 1).all() or (y < 0).any():
        return _numpy_reference(u, np.asarray(y, np.float32))
    labels = pos.argmax(axis=1)

    perm = np.argsort(labels, kind="stable")
    labels_s = labels[perm]
    counts = np.bincount(labels_s, minlength=labels_s.max() + 1)
    starts = np.concatenate([[0], np.cumsum(counts)])
    seg_s = starts[labels_s]
    seg_e = starts[labels_s + 1]
    maxn = int(counts.max())

    pad = 128
    while maxn - 1 > pad:
        pad += 128
    win = 128 + 2 * pad
    bw = ROWS_PER_CORE + 2 * pad

    if pad not in _PROGRAM_CACHE:
        _PROGRAM_CACHE[pad] = _build_program(pad)
    nc, win_, bw_ = _PROGRAM_CACHE[pad]
    assert (win_, bw_) == (win, bw)

    phi16 = np.tanh(u / 2.0).astype(np.float16)
    phiT = np.ascontiguousarray(phi16[perm].T)           # [64, N] f16, sorted
    phi64 = phiT.T.astype(np.float64)

    s_all = phi64.sum(axis=0)
    t_row = (phi64 @ s_all) / 8.0                        # sum_j t_ij incl diag
    t_diag = (phi64 * phi64).sum(axis=1) / 8.0
    w_diag = 8.0 - t_diag
    T_all = 8.0 * N - t_row

    ncls = len(counts)
    cls_sums = np.zeros((ncls, NBIT))
    np.add.at(cls_sums, labels_s, phi64)
    nseg = (seg_e - seg_s).astype(np.float64)
    Sp = nseg - 1.0
    Tp = 8.0 * Sp - ((phi64 * (cls_sums[labels_s] - phi64)).sum(axis=1)) / 8.0

    in_maps = []
    for core in range(NCORES):
        off = core * ROWS_PER_CORE
        lo = off - pad
        band = np.zeros((NBIT, bw), dtype=np.float16)
        c0 = max(0, lo)
        c1 = min(N, off + ROWS_PER_CORE + pad)
        band[:, c0 - lo : c1 - lo] = phiT[:, c0:c1]

        am = np.zeros((BLOCKS, 128, win), dtype=np.float16)
        idx = np.arange(win)[None, :]
        for blk in range(BLOCKS):
            w0 = off + 128 * blk - pad
            rows = np.arange(off + 128 * blk, off + 128 * (blk + 1))
            xs = seg_s[rows] - w0
            xe = seg_e[rows] - w0
            assert (xs >= 0).all() and (xe <= win).all(), "segment outside window"
            inside = (idx >= xs[:, None]) & (idx < xe[:, None])
            am[blk][inside] = 1.0
            am[blk, np.arange(128), rows - w0] = 0.0     # exclude diagonal
        in_maps.append({"phiT": phiT, "bandT": band, "amask": am})

    try:
        return _postprocess_and_loss(
            nc, in_maps, seg_s, seg_e, pad, T_all, Tp, Sp, w_diag
        )
    except _RangeViolation:
        return _numpy_reference(u, np.asarray(y, np.float32))


def _postprocess_and_loss(nc, in_maps, seg_s, seg_e, pad, T_all, Tp, Sp, w_diag):
    global _LAST_RESULTS
    res = run_bass_kernel_spmd(nc, in_maps, list(range(NCORES)))
    _LAST_RESULTS = res

    win = 128 + 2 * pad
    pDCp = np.zeros((N, NBINS))
    pDCn = np.zeros((N, NBINS))
    for core in range(NCORES):
        out = res.results[core]
        racc = out["racc"].astype(np.float64)            # [8, 128, 32]
        off = core * ROWS_PER_CORE
        rows = np.arange(off, off + ROWS_PER_CORE)

        R_all = np.zeros((ROWS_PER_CORE, 3))
        for ti, c in enumerate(TAPS):
            a = float(8 - c)
            acc = np.zeros(ROWS_PER_CORE)
            for sg in range(NSG):
                acc += racc[:, :, 8 * ti + 2 * sg].reshape(-1)          # ACT
                acc += a * HW_ - racc[:, :, 8 * ti + 2 * sg + 1].reshape(-1) / 8.0
            R_all[:, ti] = acc

        wd = w_diag[rows]
        Td = T_all[rows]
        L7 = R_all[:, 0] + 7.0 * N - Td                  # sum relu(7-w) incl diag
        exc7 = L7 - np.maximum(7.0 - wd, 0.0)
        R9p = R_all[:, 2] - np.maximum(wd - 9.0, 0.0)
        if (exc7 > 0.5).any() or (R9p > 0.5).any() or (exc7 < -0.5).any():
            raise _RangeViolation()

        Rt = np.zeros((ROWS_PER_CORE, 18))               # c = -1 .. 16
        Tdp = Td - wd
        for c in range(-1, 7):
            Rt[:, c + 1] = Tdp - float(c) * (N - 1)
        for ti, c in enumerate(TAPS):
            Rt[:, c + 1] = R_all[:, ti] - np.maximum(wd - c, 0.0)
        H_all = Rt[:, 0:16] - 2.0 * Rt[:, 1:17] + Rt[:, 2:18]
        H_all[:, :6] = 0.0
        H_all[:, 11:] = 0.0
        H_all = np.maximum(H_all, 0.0)

        Rb = np.zeros((ROWS_PER_CORE, 18))
        Spr = Sp[rows]
        Tpr = Tp[rows]
        for c in range(-1, 7):
            Rb[:, c + 1] = Tpr - float(c) * Spr
        for ti, c in enumerate(TAPS):
            a = float(8 - c)
            Rb[:, c + 1] = a * Spr - racc[:, :, 24 + ti].reshape(-1) / 8.0
        H_p = Rb[:, 0:16] - 2.0 * Rb[:, 1:17] + Rb[:, 2:18]
        H_p[:, :6] = 0.0
        H_p[:, 11:] = 0.0
        H_p = np.maximum(H_p, 0.0)

        H_n = np.maximum(H_all - H_p, 0.0)
        bins = np.arange(NBINS)[None, :]
        H_n += _hat(wd[:, None] - bins)                  # diagonal (xn_ii = 1)
        pDCp[rows] = H_p
        pDCn[rows] = H_n

    prCp = Sp / (N - 1)
    prCn = 1.0 - prCp
    return _finish_loss(pDCp, pDCn, prCp, prCn, N)
